# revision 1
# baseline (speedup 1.0000x reference)
"""Trainium2 Bass kernel for ComprehensiveWindowAwareLoss.

Self-contained: hardcodes shapes [16,3,512,512] f32, shards batch across 8
NeuronCores (2 images/core). Host converts inputs to fp16 and lays each
image-channel out as [128, 2048] (h = hb*128 + p, free = (hb, w)).

Per-image device pipeline (fp16 maps [128,2048] unless noted):
  window mask m = sigmoid(BR_S*v+BR_B) * sigmoid(LS_S*dsat+LS_B)
  W-pool: padded cumsum (tensor_tensor_scan) + shifted subtract
  H-pool: PE band-matrix matmuls -> PSUM; ACT copy->SBUF fp16 (+accum = SWM)
  L1:     D = |p0-t0|+|p1-t1|+|p2-t2|  (accum = SD)
  color:  G = 2*D + (0.5*|spsp-g| - stsp)/g,  g = sqrt(stst*spsp)
  SWG accum = sum(G*wm)
Host combines: total = 4/(3N)*SD + 2/(225N)*(SWM+SWG).
Rescale of source happens ONLY inside the window detector (reference
semantics); it is folded into the sigmoid scale/bias constants.
"""
import numpy as np

B, C, H, W = 16, 3, 512, 512
NCORES = 8
BPC = B // NCORES      # images per core
HB = H // 128          # 4 h-blocks
FD = HB * W            # 2048 free elems per map tile
K1 = 0.587 / 0.299
K2 = 0.114 / 0.299
N_TOT = B * H * W

_COMPILED = {}


def _band_matrices():
    k = np.arange(128)[:, None]
    m = np.arange(128)[None, :]
    B_mid = (np.abs(k - m) <= 7).astype(np.float16)
    B_up = ((k - m >= 121) & (k - m <= 135)).astype(np.float16)
    B_dn = ((m - k >= 121) & (m - k <= 135)).astype(np.float16)
    return np.stack([B_mid, B_up, B_dn])  # [3,128,128]


def _build(br_s, br_b, ls_s, ls_b):
    import concourse.bass as bass
    import concourse.bacc as bacc
    import concourse.tile as tile
    from concourse import mybir

    f16 = mybir.dt.float16
    f32 = mybir.dt.float32
    Alu = mybir.AluOpType
    Act = mybir.ActivationFunctionType

    nc = bacc.Bacc("TRN2", debug=False, num_devices=NCORES)
    p_d = nc.dram_tensor("p", [BPC, C, 128, FD], f16, kind="ExternalInput").ap()
    t_d = nc.dram_tensor("t", [BPC, C, 128, FD], f16, kind="ExternalInput").ap()
    a_d = nc.dram_tensor("a", [BPC, C, 128, FD], f16, kind="ExternalInput").ap()
    k_d = nc.dram_tensor("k", [3, 128, 128], f16, kind="ExternalInput").ap()
    o_d = nc.dram_tensor("o", [128, 16], f32, kind="ExternalOutput").ap()

    with tile.TileContext(nc) as tc:
        with (
            tc.tile_pool(name="inp", bufs=2) as inp,
            tc.tile_pool(name="wk", bufs=1) as wk,
            tc.tile_pool(name="cst", bufs=1) as cst,
            tc.tile_pool(name="ps", bufs=4, space=bass.MemorySpace.PSUM) as ps,
        ):
            kt = cst.tile([128, 3, 128], f16, tag="bands")
            for j in range(3):
                nc.sync.dma_start(kt[:, j, :], k_d[j])
            part = cst.tile([128, 16], f32, tag="part")
            nc.vector.memset(part[:], 0.0)
            b_br = cst.tile([128, 1], f32, tag="b_br")
            nc.vector.memset(b_br[:], br_b)
            b_ls = cst.tile([128, 1], f32, tag="b_ls")
            nc.vector.memset(b_ls[:], ls_b)
            b_eps = cst.tile([128, 1], f32, tag="b_eps")
            nc.vector.memset(b_eps[:], 1e-12)

            for img in range(BPC):
                base = img * 8
                a = [inp.tile([128, FD], f16, tag=f"a{c}", name=f"a{c}") for c in range(C)]
                for c in range(C):
                    nc.sync.dma_start(a[c][:], a_d[img, c])

                # ---- window mask ----
                u = wk.tile([128, FD], f16, tag="w0", bufs=2)
                nc.vector.scalar_tensor_tensor(u[:], a[1][:], K1, a[0][:], Alu.mult, Alu.add)
                v = wk.tile([128, FD], f16, tag="w1", bufs=2)
                nc.vector.scalar_tensor_tensor(v[:], a[2][:], K2, u[:], Alu.mult, Alu.add)
                bright = wk.tile([128, FD], f16, tag="w2", bufs=2)
                nc.scalar.activation(bright[:], v[:], Act.Sigmoid, bias=b_br[:], scale=br_s)
                mx = wk.tile([128, FD], f16, tag="w0", bufs=2)
                nc.vector.tensor_max(mx[:], a[0][:], a[1][:])
                mx2 = wk.tile([128, FD], f16, tag="w1", bufs=2)
                nc.vector.tensor_max(mx2[:], mx[:], a[2][:])
                mn = wk.tile([128, FD], f16, tag="w3")
                nc.vector.tensor_tensor(mn[:], a[0][:], a[1][:], Alu.min)
                mn2 = wk.tile([128, FD], f16, tag="w4")
                nc.vector.tensor_tensor(mn2[:], mn[:], a[2][:], Alu.min)
                dsat = wk.tile([128, FD], f16, tag="w0", bufs=2)
                nc.vector.tensor_sub(dsat[:], mx2[:], mn2[:])
                lowsat = wk.tile([128, FD], f16, tag="w1", bufs=2)
                nc.scalar.activation(lowsat[:], dsat[:], Act.Sigmoid, bias=b_ls[:], scale=ls_s)

                mpad = wk.tile([128, HB * 528], f16, tag="mpad")
                mp3 = mpad[:].rearrange("p (b w) -> p b w", b=HB)
                nc.vector.memset(mp3[:, :, 0:8], 0.0)
                nc.vector.memset(mp3[:, :, 520:528], 0.0)
                nc.vector.tensor_mul(mp3[:, :, 8:520], bright[:], lowsat[:])

                # ---- W-pool: cumsum + shifted subtract ----
                cs = wk.tile([128, HB * 528], f16, tag="csum")
                nc.vector.tensor_tensor_scan(
                    cs[:], mpad[:], mpad[:], 0.0, Alu.add, Alu.bypass
                )
                c3 = cs[:].rearrange("p (b w) -> p b w", b=HB)
                pw = wk.tile([128, HB, W], f16, tag="w2", bufs=2)
                nc.vector.tensor_sub(pw[:], c3[:, :, 15:527], c3[:, :, 0:512])

                # ---- H-pool: PE band matmuls + ACT copy (accum -> SWM) ----
                wm = wk.tile([128, FD], f16, tag="wm", bufs=2)
                for hb in range(HB):
                    terms = [(1, hb)]
                    if hb > 0:
                        terms.append((2, hb - 1))
                    if hb < HB - 1:
                        terms.append((0, hb + 1))
                    # note: k_d[0]=B_mid? host packs [B_mid,B_up,B_dn]; index map below
                    acc = ps.tile([128, W], f32, tag="psum", name=f"acc{hb}")
                    for i, (mat, src) in enumerate(terms):
                        nc.tensor.matmul(
                            acc[:],
                            kt[:, {1: 0, 2: 1, 0: 2}[mat], :],
                            pw[:, src, :],
                            start=(i == 0),
                            stop=(i == len(terms) - 1),
                        )
                    nc.scalar.activation(
                        wm[:, hb * W : (hb + 1) * W],
                        acc[:],
                        Act.Identity,
                        accum_out=part[:, base + hb : base + hb + 1],
                    )

                # ---- L1 ----
                p = [inp.tile([128, FD], f16, tag=f"p{c}", name=f"p{c}") for c in range(C)]
                t = [inp.tile([128, FD], f16, tag=f"t{c}", name=f"t{c}") for c in range(C)]
                for c in range(C):
                    nc.sync.dma_start(p[c][:], p_d[img, c])
                    nc.sync.dma_start(t[c][:], t_d[img, c])

                d = []
                for c in range(C):
                    dc = wk.tile([128, FD], f16, tag=f"d{c}", name=f"dc{c}")
                    nc.vector.tensor_sub(dc[:], p[c][:], t[c][:])
                    d.append(dc)
                e = []
                for c in range(C):
                    ec = wk.tile([128, FD], f16, tag=f"e{c}", name=f"ec{c}")
                    nc.scalar.activation(ec[:], d[c][:], Act.Abs)
                    e.append(ec)
                d01 = wk.tile([128, FD], f16, tag="w0", bufs=2)
                nc.vector.tensor_add(d01[:], e[0][:], e[1][:])
                D = wk.tile([128, FD], f16, tag="D", bufs=2)
                nc.vector.scalar_tensor_tensor(
                    D[:], e[2][:], 0.0, d01[:], Alu.add, Alu.add,
                    accum_out=part[:, base + 4 : base + 5],
                )

                # ---- color: st/sp gram ----
                st = []
                sp = []
                for c in range(C):
                    stc = wk.tile([128, FD], f16, tag=f"e{c}", name=f"stc{c}")  # e_c dead
                    nc.vector.tensor_sub(stc[:], t[c][:], a[c][:])
                    st.append(stc)
                    spc = wk.tile([128, FD], f16, tag=f"d{c}", name=f"spc{c}")  # d_c dead
                    nc.vector.tensor_sub(spc[:], p[c][:], a[c][:])
                    sp.append(spc)

                def csum3(maker, tag1, tag2, tag3, accum=None):
                    q = []
                    for c in range(C):
                        qc = wk.tile([128, FD], f16, tag=f"{tag1}{c}", name=f"{tag1}q{c}")
                        maker(qc, c)
                        q.append(qc)
                    s01 = wk.tile([128, FD], f16, tag=tag2, name=f"s01_{tag3}", bufs=2)
                    nc.vector.tensor_add(s01[:], q[0][:], q[1][:])
                    out = wk.tile([128, FD], f16, tag=tag3, name=tag3, bufs=2)
                    nc.vector.tensor_add(out[:], s01[:], q[2][:])
                    return out

                stsp = csum3(
                    lambda qc, c: nc.vector.tensor_mul(qc[:], st[c][:], sp[c][:]),
                    "q", "w0", "stsp",
                )
                stst = csum3(
                    lambda qc, c: nc.scalar.activation(qc[:], st[c][:], Act.Square),
                    "q", "w0", "stst",
                )
                spsp = csum3(
                    lambda qc, c: nc.scalar.activation(qc[:], sp[c][:], Act.Square),
                    "q", "w0", "spsp",
                )

                gp = wk.tile([128, FD], f16, tag="w0", bufs=2)
                nc.vector.tensor_mul(gp[:], stst[:], spsp[:])
                g32 = wk.tile([128, FD], f32, tag="g32")
                nc.scalar.activation(g32[:], gp[:], Act.Sqrt, bias=b_eps[:])
                g16 = wk.tile([128, FD], f16, tag="w1", bufs=2)
                nc.scalar.activation(g16[:], gp[:], Act.Sqrt, bias=b_eps[:])
                rg = wk.tile([128, FD], f32, tag="rg32")
                nc.vector.reciprocal_approx_fast(rg[:], g32[:])
                snum = wk.tile([128, FD], f16, tag="w2", bufs=2)
                nc.vector.tensor_sub(snum[:], spsp[:], g16[:])
                anum = wk.tile([128, FD], f16, tag="w1", bufs=2)
                nc.scalar.activation(anum[:], snum[:], Act.Abs)
                numer = wk.tile([128, FD], f16, tag="w2", bufs=2)
                nc.vector.scalar_tensor_tensor(
                    numer[:], anum[:], 0.5, stsp[:], Alu.mult, Alu.subtract
                )
                nr = wk.tile([128, FD], f16, tag="w0", bufs=2)
                nc.vector.tensor_mul(nr[:], numer[:], rg[:])
                G = wk.tile([128, FD], f16, tag="w1", bufs=2)
                nc.vector.scalar_tensor_tensor(G[:], D[:], 2.0, nr[:], Alu.mult, Alu.add)
                scr = wk.tile([128, FD], f16, tag="w2", bufs=2)
                nc.vector.scalar_tensor_tensor(
                    scr[:], G[:], 0.0, wm[:], Alu.add, Alu.mult,
                    accum_out=part[:, base + 5 : base + 6],
                )

            nc.sync.dma_start(o_d[:], part[:])

    nc.compile()
    return nc


def _get_nc(rescale):
    key = bool(rescale)
    if key not in _COMPILED:
        cs, cb = (0.5, 0.5) if rescale else (1.0, 0.0)
        _COMPILED[key] = _build(
            20.0 * 0.299 * cs, 20.0 * (cb - 0.65), -20.0 * cs, 20.0 * 0.15
        )
    return _COMPILED[key]


def _host_layout(x16):
    # [B,C,H,W] fp16 -> per-core [BPC,C,128,FD] with h = hb*128 + p
    xs = x16.reshape(NCORES, BPC, C, HB, 128, W).transpose(0, 1, 2, 4, 3, 5)
    return np.ascontiguousarray(xs.reshape(NCORES, BPC, C, 128, FD))


def kernel(pred, target, source, _trace=False):
    from concourse.bass_utils import run_bass_kernel_spmd

    rescale = bool(source.min() < 0)
    nc = _get_nc(rescale)

    p = _host_layout(pred.astype(np.float16))
    t = _host_layout(target.astype(np.float16))
    a = _host_layout(source.astype(np.float16))
    k = _band_matrices()

    in_maps = [
        {"p": p[i], "t": t[i], "a": a[i], "k": k} for i in range(NCORES)
    ]
    res = run_bass_kernel_spmd(
        nc, in_maps, core_ids=list(range(NCORES)), trace=_trace
    )
    parts = np.stack([r["o"] for r in res.results])  # [8,128,16]
    ps = parts.sum(axis=(0, 1), dtype=np.float64)    # [16]
    swm = ps[0:4].sum() + ps[8:12].sum()
    sd = ps[4] + ps[12]
    swg = ps[5] + ps[13]
    total = (4.0 / (3 * N_TOT)) * sd + (2.0 / (225.0 * N_TOT)) * (swm + swg)
    out = np.float32(total)
    if _trace:
        return out, res
    return out



# revision 14
# speedup vs baseline: 1.2666x; 1.2666x over previous
"""Trainium2 Bass kernel for ComprehensiveWindowAwareLoss (v5).

Self-contained: hardcodes shapes [16,3,512,512] f32, shards batch across 8
NeuronCores (2 images/core). Host converts inputs to fp16 and lays each
image out as a [128, 6144] slab: free = (c, hb, w), h = hb*128 + p.

Engine plan (HW-validated ops only; Pool/GpSimd has no elementwise support
in this compile pipeline):
  DVE:  mask chain (u/v brightness combine, max3/min3/dsat, mprod,
        W-pool cumsum + shifted subtract) and the color/L1 chunks
        (d, st, sp, q=st*sp, channel sums, gp, wm tails + ts-accums)
  Act:  sigmoids, e=|d| (Abs, accum -> SD), st^2/sp^2 squares,
        |spsp*h-wm| (Abs, accum -> S_yw), per-hb wm copies (accum S_m),
        rsqrt(stst*spsp+eps) via direct InstActivation
  PE:   H-pool band matmuls (one PSUM bank per hb)
Host: total = 4/(3N)*SD + 2/(225N)*(S_m + 2*S_Dw + 0.5*S_yw - S_ww).
Source rescale ((x+1)/2 when min<0) is folded into sigmoid constants.
Emission order is hand-scheduled (engines execute in-order).
"""
import numpy as np

B, C, H, W = 16, 3, 512, 512
NCORES = 8
BPC = B // NCORES      # images per core
HB = H // 128          # 4 h-blocks
FD = HB * W            # 2048 free elems per channel map
SLAB = C * FD          # 6144 free elems per slab
K1 = 0.587 / 0.299
K2 = 0.114 / 0.299
N_TOT = B * H * W
EPS_G = 4e-3           # eps inside rsqrt(stst*spsp + eps); keeps fp16 safe
NSLOT = 48             # accumulator slots: (img, hb) x 6

_COMPILED = {}


def _band_matrices():
    k = np.arange(128)[:, None]
    m = np.arange(128)[None, :]
    B_mid = (np.abs(k - m) <= 7).astype(np.float16)
    B_up = ((k - m >= 121) & (k - m <= 135)).astype(np.float16)
    B_dn = ((m - k >= 121) & (m - k <= 135)).astype(np.float16)
    return np.stack([B_mid, B_up, B_dn])  # [3,128,128]


def _build(br_s, br_b, ls_s, ls_b):
    import concourse.bass as bass
    import concourse.bacc as bacc
    import concourse.tile as tile
    from concourse import mybir

    f16 = mybir.dt.float16
    u16 = mybir.dt.uint16
    f32 = mybir.dt.float32
    Alu = mybir.AluOpType
    Act = mybir.ActivationFunctionType

    nc = bacc.Bacc("TRN2", debug=False, num_devices=NCORES)

    def act_rsqrt(out, in_, bias_ap):
        # nc.scalar.activation refuses Rsqrt (HW accuracy ~3e-4 measured;
        # fine at our tolerance); emit the InstActivation directly.
        eng = nc.scalar
        return eng.add_instruction(
            mybir.InstActivation(
                name=nc.get_next_instruction_name(),
                func=Act.Rsqrt,
                ins=[
                    eng.lower_ap(in_),
                    eng.lower_ap(bias_ap),
                    mybir.ImmediateValue(dtype=f32, value=1.0),
                    mybir.ImmediateValue(dtype=f32, value=0.0),
                ],
                outs=[eng.lower_ap(out)],
            )
        )

    p_d = nc.dram_tensor("p", [BPC, 128, SLAB], f16, kind="ExternalInput").ap()
    t_d = nc.dram_tensor("t", [BPC, 128, SLAB], f16, kind="ExternalInput").ap()
    a_d = nc.dram_tensor("a", [BPC, 128, SLAB], f16, kind="ExternalInput").ap()
    k_d = nc.dram_tensor("k", [3, 128, 128], f16, kind="ExternalInput").ap()
    o_d = nc.dram_tensor("o", [128, NSLOT], f32, kind="ExternalOutput").ap()

    with tile.TileContext(nc) as tc:
        with (
            tc.tile_pool(name="inp", bufs=2) as inp,
            tc.tile_pool(name="msk", bufs=1) as msk,
            tc.tile_pool(name="wk", bufs=2) as wk,
            tc.tile_pool(name="cst", bufs=1) as cst,
            tc.tile_pool(name="ps", bufs=2, space=bass.MemorySpace.PSUM) as ps,
        ):
            st_img = [{} for _ in range(BPC)]

            def dma_inputs(img, phase):
                s = st_img[img]
                if phase == 0:
                    a = inp.tile([128, SLAB], f16, tag="a", name=f"a{img}")
                    s["a"] = a
                    a3 = a_d[img].rearrange("p (c f) -> p c f", c=C)
                    nc.sync.dma_start(a[:, 0 * FD:1 * FD], a3[:, 0, :])
                    nc.sync.dma_start(a[:, 1 * FD:2 * FD], a3[:, 1, :])
                    p = inp.tile([128, SLAB], f16, tag="p", name=f"p{img}")
                    t = inp.tile([128, SLAB], f16, tag="t", name=f"t{img}")
                    s["p"], s["t"] = p, t
                    pr = p_d[img].rearrange("p (c b f) -> p c b f", c=C, b=HB)
                    tr = t_d[img].rearrange("p (c b f) -> p c b f", c=C, b=HB)
                    p3 = p[:].rearrange("p (c b f) -> p c b f", c=C, b=HB)
                    t3 = t[:].rearrange("p (c b f) -> p c b f", c=C, b=HB)
                    s["pr"], s["tr"], s["p3"], s["t3"] = pr, tr, p3, t3
                    nc.sync.dma_start(a[:, 2 * FD:3 * FD], a3[:, 2, :])
                    nc.sync.dma_start(p3[:, :, 0, :], pr[:, :, 0, :])
                    nc.sync.dma_start(t3[:, :, 0, :], tr[:, :, 0, :])
                else:
                    for hb in range(1, HB):
                        nc.sync.dma_start(
                            s["p3"][:, :, hb, :], s["pr"][:, :, hb, :])
                        nc.sync.dma_start(
                            s["t3"][:, :, hb, :], s["tr"][:, :, hb, :])

            def mask_front(img):
                s = st_img[img]
                a = s["a"]
                ac = [a[:, c * FD:(c + 1) * FD] for c in range(C)]
                u0 = msk.tile([128, FD], f16, tag="u0", name=f"u0_{img}")
                nc.vector.tensor_scalar(u0[:], ac[1], K1, None, Alu.mult)
                u = msk.tile([128, FD], f16, tag="u", name=f"u{img}")
                nc.vector.tensor_add(u[:], u0[:], ac[0])
                v0 = msk.tile([128, FD], f16, tag="u0", name=f"v0_{img}")
                nc.vector.tensor_scalar(v0[:], ac[2], K2, None, Alu.mult)
                v = msk.tile([128, FD], f16, tag="v", name=f"v{img}")
                nc.vector.tensor_add(v[:], v0[:], u[:])
                bright = msk.tile([128, FD], f16, tag="u", name=f"bright{img}")
                nc.scalar.activation(
                    bright[:], v[:], Act.Sigmoid, bias=s["b_br"], scale=br_s)
                s["bright"] = bright
                mx = msk.tile([128, FD], f16, tag="mx", name=f"mx{img}")
                nc.vector.tensor_max(mx[:], ac[0], ac[1])
                mx2 = msk.tile([128, FD], f16, tag="mx2", name=f"mx2{img}")
                nc.vector.tensor_max(mx2[:], mx[:], ac[2])
                mn = msk.tile([128, FD], f16, tag="mn", name=f"mn{img}")
                nc.vector.tensor_tensor(mn[:], ac[0], ac[1], Alu.min)
                mn2 = msk.tile([128, FD], f16, tag="mn2", name=f"mn2{img}")
                nc.vector.tensor_tensor(mn2[:], mn[:], ac[2], Alu.min)
                dsat = msk.tile([128, FD], f16, tag="mx", name=f"dsat{img}")
                nc.vector.tensor_sub(dsat[:], mx2[:], mn2[:])
                s["dsat"] = dsat

            def mask_back(img):
                s = st_img[img]
                lowsat = msk.tile([128, FD], f16, tag="mn",
                                  name=f"lowsat{img}")
                nc.scalar.activation(
                    lowsat[:], s["dsat"][:], Act.Sigmoid,
                    bias=s["b_ls"], scale=ls_s)
                mpad = msk.tile([128, HB * 528], f16, tag="mpad",
                                name=f"mpad{img}")
                mp3 = mpad[:].rearrange("p (b w) -> p b w", b=HB)
                nc.vector.memset(mp3[:, :, 0:8], 0.0)
                nc.vector.memset(mp3[:, :, 520:528], 0.0)
                nc.vector.tensor_mul(mp3[:, :, 8:520], s["bright"][:],
                                     lowsat[:])
                cs = msk.tile([128, HB * 528], f16, tag="csum",
                              name=f"cs{img}")
                nc.vector.tensor_tensor_scan(
                    cs[:], mpad[:], mpad[:], 0.0, Alu.add, Alu.bypass)
                c3 = cs[:].rearrange("p (b w) -> p b w", b=HB)
                pw = msk.tile([128, HB, W], f16, tag="mpad", name=f"pw{img}")
                nc.vector.tensor_sub(pw[:], c3[:, :, 15:527],
                                     c3[:, :, 0:512])
                acc = ps.tile([128, HB, W], f32, tag="psum", name=f"acc{img}")
                s["acc"] = acc
                for hb in range(HB):
                    terms = [(1, hb)]
                    if hb > 0:
                        terms.append((2, hb - 1))
                    if hb < HB - 1:
                        terms.append((0, hb + 1))
                    for i, (mat, src) in enumerate(terms):
                        nc.tensor.matmul(
                            acc[:, hb, :],
                            s["kt"][:, {1: 0, 2: 1, 0: 2}[mat], :],
                            pw[:, src, :],
                            start=(i == 0),
                            stop=(i == len(terms) - 1),
                        )

            def chunk_a(img, hb):
                s = st_img[img]
                base = (img * HB + hb) * 6
                part = s["part"]
                sl = slice(hb * W, hb * W + W)
                p3 = s["p"][:].rearrange("p (c f) -> p c f", c=C)[:, :, sl]
                t3 = s["t"][:].rearrange("p (c f) -> p c f", c=C)[:, :, sl]
                ah = s["a"][:].rearrange("p (c f) -> p c f", c=C)[:, :, sl]

                d = wk.tile([128, C, W], f16, tag="d", name=f"d{img}_{hb}")
                nc.vector.tensor_sub(d[:], p3, t3)
                e = wk.tile([128, C, W], f16, tag="e", name=f"e{img}_{hb}")
                nc.scalar.activation(
                    e[:], d[:], Act.Abs,
                    accum_out=part[:, base + 1:base + 2])
                st = wk.tile([128, C, W], f16, tag="st", name=f"st{img}_{hb}")
                nc.vector.tensor_sub(st[:], t3, ah)
                sp = wk.tile([128, C, W], f16, tag="sp", name=f"sp{img}_{hb}")
                nc.vector.tensor_add(sp[:], st[:], d[:])
                r = wk.tile([128, C, W], f16, tag="r", name=f"r{img}_{hb}")
                nc.scalar.activation(r[:], st[:], Act.Square)
                r2 = wk.tile([128, C, W], f16, tag="r2", name=f"r2{img}_{hb}")
                nc.scalar.activation(r2[:], sp[:], Act.Square)
                q = wk.tile([128, C, W], f16, tag="q", name=f"q{img}_{hb}")
                nc.vector.tensor_mul(q[:], st[:], sp[:])
                Dt = wk.tile([128, W], f16, tag="Dt", name=f"Dt{img}_{hb}")
                nc.vector.tensor_add(Dt[:], e[:, 0, :], e[:, 1, :])
                D = wk.tile([128, W], f16, tag="D", name=f"DD{img}_{hb}")
                nc.vector.tensor_add(D[:], Dt[:], e[:, 2, :])
                s1 = wk.tile([128, W], f16, tag="s1", name=f"s1_{img}_{hb}")
                nc.vector.tensor_add(s1[:], q[:, 0, :], q[:, 1, :])
                stsp = wk.tile([128, W], f16, tag="stsp",
                               name=f"stsp{img}_{hb}")
                nc.vector.tensor_add(stsp[:], s1[:], q[:, 2, :])
                s2 = wk.tile([128, W], f16, tag="s2", name=f"s2_{img}_{hb}")
                nc.vector.tensor_add(s2[:], r[:, 0, :], r[:, 1, :])
                stst = wk.tile([128, W], f16, tag="stst",
                               name=f"stst{img}_{hb}")
                nc.vector.tensor_add(stst[:], s2[:], r[:, 2, :])
                s3 = wk.tile([128, W], f16, tag="s3", name=f"s3_{img}_{hb}")
                nc.vector.tensor_add(s3[:], r2[:, 0, :], r2[:, 1, :])
                spsp = wk.tile([128, W], f16, tag="spsp",
                               name=f"spsp{img}_{hb}")
                nc.vector.tensor_add(spsp[:], s3[:], r2[:, 2, :])
                gp = wk.tile([128, W], f16, tag="gp", name=f"gp{img}_{hb}")
                nc.vector.tensor_mul(gp[:], stst[:], spsp[:])
                s[f"D{hb}"] = D
                s[f"stsp{hb}"] = stsp
                s[f"spsp{hb}"] = spsp
                s[f"gp{hb}"] = gp

            def wm_rg(img):
                s = st_img[img]
                for hb in range(HB):
                    base = (img * HB + hb) * 6
                    wm = wk.tile([128, W], f16, tag="wm", name=f"wm{img}_{hb}")
                    nc.scalar.activation(
                        wm[:], s["acc"][:, hb, :], Act.Identity,
                        accum_out=s["part"][:, base + 0:base + 1])
                    s[f"wm{hb}"] = wm
                for hb in range(HB):
                    rg = wk.tile([128, W], f16, tag="rg", name=f"rg{img}_{hb}")
                    act_rsqrt(rg[:], s[f"gp{hb}"][:], s["b_eps"])
                    s[f"rg{hb}"] = rg

            def tails(img):
                s = st_img[img]
                for hb in range(HB):
                    base = (img * HB + hb) * 6
                    part = s["part"]
                    wm, rg = s[f"wm{hb}"], s[f"rg{hb}"]
                    h = wk.tile([128, W], f16, tag="h", name=f"h{img}_{hb}")
                    nc.vector.tensor_mul(h[:], rg[:], wm[:])
                    q1 = wk.tile([128, W], f16, tag="q1",
                                 name=f"q1_{img}_{hb}")
                    nc.vector.tensor_mul(q1[:], s[f"spsp{hb}"][:], h[:])
                    q2 = wk.tile([128, W], f16, tag="q2",
                                 name=f"q2_{img}_{hb}")
                    nc.vector.tensor_sub(q2[:], q1[:], wm[:])
                    scr = wk.tile([128, W], f16, tag="scr",
                                  name=f"scr{img}_{hb}")
                    nc.scalar.activation(
                        scr[:], q2[:], Act.Abs,
                        accum_out=part[:, base + 2:base + 3])
                    q3 = wk.tile([128, W], f16, tag="q3",
                                 name=f"q3_{img}_{hb}")
                    nc.vector.tensor_mul(q3[:], s[f"stsp{hb}"][:], h[:])
                    nc.vector.tensor_scalar(
                        q3[:], q3[:], 1.0, None, Alu.mult, Alu.add,
                        accum_out=part[:, base + 3:base + 4])
                    Dw = wk.tile([128, W], f16, tag="Dw",
                                 name=f"Dw{img}_{hb}")
                    nc.vector.tensor_mul(Dw[:], s[f"D{hb}"][:], wm[:])
                    nc.vector.tensor_scalar(
                        Dw[:], Dw[:], 1.0, None, Alu.mult, Alu.add,
                        accum_out=part[:, base + 4:base + 5])

            # ---------- global emission order ----------
            dma_inputs(0, 0)
            kt = cst.tile([128, 3, 128], f16, tag="bands")
            for j in range(3):
                nc.sync.dma_start(kt[:, j, :], k_d[j])
            part = cst.tile([128, NSLOT], f32, tag="part")
            nc.vector.memset(part[:], 0.0)
            b_br = cst.tile([128, 1], f32, tag="b_br")
            nc.vector.memset(b_br[:], br_b)
            b_ls = cst.tile([128, 1], f32, tag="b_ls")
            nc.vector.memset(b_ls[:], ls_b)
            b_eps = cst.tile([128, 1], f32, tag="b_eps")
            nc.vector.memset(b_eps[:], EPS_G)
            for s in st_img:
                s["kt"] = kt
                s["part"] = part
                s["b_br"] = b_br[:]
                s["b_ls"] = b_ls[:]
                s["b_eps"] = b_eps[:]

            dma_inputs(0, 1)
            mask_front(0)
            chunk_a(0, 0)
            dma_inputs(1, 0)
            mask_back(0)
            chunk_a(0, 1)
            chunk_a(0, 2)
            chunk_a(0, 3)
            wm_rg(0)
            dma_inputs(1, 1)
            mask_front(1)
            tails(0)
            chunk_a(1, 0)
            mask_back(1)
            chunk_a(1, 1)
            chunk_a(1, 2)
            chunk_a(1, 3)
            wm_rg(1)
            tails(1)

            nc.sync.dma_start(o_d[:], part[:])

    nc.compile()
    return nc


def _get_nc(rescale):
    key = bool(rescale)
    if key not in _COMPILED:
        cs, cb = (0.5, 0.5) if rescale else (1.0, 0.0)
        _COMPILED[key] = _build(
            20.0 * 0.299 * cs, 20.0 * (cb * (0.299 + 0.587 + 0.114) - 0.65),
            -20.0 * cs, 20.0 * 0.15,
        )
    return _COMPILED[key]


def _host_layout(x16):
    # [B,C,H,W] fp16 -> per-core [BPC,128,SLAB]; slab free = (c, hb, w)
    xs = x16.reshape(NCORES, BPC, C, HB, 128, W).transpose(0, 1, 4, 2, 3, 5)
    return np.ascontiguousarray(xs.reshape(NCORES, BPC, 128, SLAB))


def kernel(pred, target, source, _trace=False):
    from concourse.bass_utils import run_bass_kernel_spmd

    rescale = bool(source.min() < 0)
    nc = _get_nc(rescale)

    p = _host_layout(pred.astype(np.float16))
    t = _host_layout(target.astype(np.float16))
    a = _host_layout(source.astype(np.float16))
    k = _band_matrices()

    in_maps = [
        {"p": p[i], "t": t[i], "a": a[i], "k": k} for i in range(NCORES)
    ]
    res = run_bass_kernel_spmd(
        nc, in_maps, core_ids=list(range(NCORES)), trace=_trace
    )
    parts = np.stack([r["o"] for r in res.results])  # [8,128,NSLOT]
    ps = parts.sum(axis=(0, 1), dtype=np.float64)    # [NSLOT]
    sl = ps[:BPC * HB * 6].reshape(BPC * HB, 6)
    s_m, sd, s_yw, s_ww, s_dw = (sl[:, j].sum() for j in range(5))
    total = (4.0 / (3 * N_TOT)) * sd + (2.0 / (225.0 * N_TOT)) * (
        s_m + 2.0 * s_dw + 0.5 * s_yw - s_ww
    )
    out = np.float32(total)
    if _trace:
        return out, res
    return out


# revision 19
# speedup vs baseline: 1.3261x; 1.0470x over previous
"""Trainium2 Bass kernel for ComprehensiveWindowAwareLoss (v5).

Self-contained: hardcodes shapes [16,3,512,512] f32, shards batch across 8
NeuronCores (2 images/core). Host converts inputs to fp16 and lays each
image out as a [128, 6144] slab: free = (c, hb, w), h = hb*128 + p.

Engine plan (HW-validated ops only; Pool/GpSimd has no elementwise support
in this compile pipeline):
  DVE:  mask chain (u/v brightness combine, max3/min3/dsat, mprod,
        W-pool cumsum + shifted subtract) and the color/L1 chunks
        (d, st, sp, q=st*sp, channel sums, gp, wm tails + ts-accums)
  Act:  sigmoids, e=|d| (Abs, accum -> SD), st^2/sp^2 squares,
        |spsp*h-wm| (Abs, accum -> S_yw), per-hb wm copies (accum S_m),
        rsqrt(stst*spsp+eps) via direct InstActivation
  PE:   H-pool band matmuls (one PSUM bank per hb)
Host: total = 4/(3N)*SD + 2/(225N)*(S_m + 2*S_Dw + 0.5*S_yw - S_ww).
Source rescale ((x+1)/2 when min<0) is folded into sigmoid constants.
Emission order is hand-scheduled (engines execute in-order).
"""
import numpy as np

B, C, H, W = 16, 3, 512, 512
NCORES = 8
BPC = B // NCORES      # images per core
HB = H // 128          # 4 h-blocks
FD = HB * W            # 2048 free elems per channel map
SLAB = C * FD          # 6144 free elems per slab
K1 = 0.587 / 0.299
K2 = 0.114 / 0.299
N_TOT = B * H * W
EPS_G = 4e-3           # eps inside rsqrt(stst*spsp + eps); keeps fp16 safe
NSLOT = 48             # accumulator slots: (img, hb) x 6

_COMPILED = {}


def _band_matrices():
    k = np.arange(128)[:, None]
    m = np.arange(128)[None, :]
    B_mid = (np.abs(k - m) <= 7).astype(np.float16)
    B_up = ((k - m >= 121) & (k - m <= 135)).astype(np.float16)
    B_dn = ((m - k >= 121) & (m - k <= 135)).astype(np.float16)
    return np.stack([B_mid, B_up, B_dn])  # [3,128,128]


def _build(br_s, br_b, ls_s, ls_b):
    import concourse.bass as bass
    import concourse.bacc as bacc
    import concourse.tile as tile
    from concourse import mybir

    f16 = mybir.dt.float16
    u16 = mybir.dt.uint16
    f32 = mybir.dt.float32
    Alu = mybir.AluOpType
    Act = mybir.ActivationFunctionType

    nc = bacc.Bacc("TRN2", debug=False, num_devices=NCORES)

    def act_rsqrt(out, in_, bias_ap):
        # nc.scalar.activation refuses Rsqrt (HW accuracy ~3e-4 measured;
        # fine at our tolerance); emit the InstActivation directly.
        eng = nc.scalar
        return eng.add_instruction(
            mybir.InstActivation(
                name=nc.get_next_instruction_name(),
                func=Act.Rsqrt,
                ins=[
                    eng.lower_ap(in_),
                    eng.lower_ap(bias_ap),
                    mybir.ImmediateValue(dtype=f32, value=1.0),
                    mybir.ImmediateValue(dtype=f32, value=0.0),
                ],
                outs=[eng.lower_ap(out)],
            )
        )

    p_d = nc.dram_tensor("p", [BPC, 128, SLAB], f16, kind="ExternalInput").ap()
    t_d = nc.dram_tensor("t", [BPC, 128, SLAB], f16, kind="ExternalInput").ap()
    a_d = nc.dram_tensor("a", [BPC, 128, SLAB], f16, kind="ExternalInput").ap()
    k_d = nc.dram_tensor("k", [3, 128, 128], f16, kind="ExternalInput").ap()
    o_d = nc.dram_tensor("o", [128, NSLOT], f32, kind="ExternalOutput").ap()

    with tile.TileContext(nc) as tc:
        with (
            tc.tile_pool(name="inp", bufs=2) as inp,
            tc.tile_pool(name="msk", bufs=1) as msk,
            tc.tile_pool(name="wk", bufs=2) as wk,
            tc.tile_pool(name="cst", bufs=1) as cst,
            tc.tile_pool(name="ps", bufs=2, space=bass.MemorySpace.PSUM) as ps,
        ):
            st_img = [{} for _ in range(BPC)]

            def dma_inputs(img, phase):
                s = st_img[img]
                if phase == 0:
                    a = inp.tile([128, SLAB], f16, tag="a", name=f"a{img}")
                    s["a"] = a
                    a3 = a_d[img].rearrange("p (c f) -> p c f", c=C)
                    nc.sync.dma_start(a[:, 0 * FD:1 * FD], a3[:, 0, :])
                    nc.sync.dma_start(a[:, 1 * FD:2 * FD], a3[:, 1, :])
                    p = inp.tile([128, SLAB], f16, tag="p", name=f"p{img}")
                    t = inp.tile([128, SLAB], f16, tag="t", name=f"t{img}")
                    s["p"], s["t"] = p, t
                    pr = p_d[img].rearrange("p (c b f) -> p c b f", c=C, b=HB)
                    tr = t_d[img].rearrange("p (c b f) -> p c b f", c=C, b=HB)
                    p3 = p[:].rearrange("p (c b f) -> p c b f", c=C, b=HB)
                    t3 = t[:].rearrange("p (c b f) -> p c b f", c=C, b=HB)
                    s["pr"], s["tr"], s["p3"], s["t3"] = pr, tr, p3, t3
                    nc.sync.dma_start(a[:, 2 * FD:3 * FD], a3[:, 2, :])
                    nc.sync.dma_start(p3[:, :, 0, :], pr[:, :, 0, :])
                    nc.sync.dma_start(t3[:, :, 0, :], tr[:, :, 0, :])
                else:
                    for hb in range(1, HB):
                        nc.sync.dma_start(
                            s["p3"][:, :, hb, :], s["pr"][:, :, hb, :])
                        nc.sync.dma_start(
                            s["t3"][:, :, hb, :], s["tr"][:, :, hb, :])

            def mask_front(img):
                s = st_img[img]
                a = s["a"]
                ac = [a[:, c * FD:(c + 1) * FD] for c in range(C)]
                u0 = msk.tile([128, FD], f16, tag="u0", name=f"u0_{img}")
                nc.scalar.activation(u0[:], ac[1], Act.Copy, bias=0.0,
                                     scale=K1)
                u = msk.tile([128, FD], f16, tag="u", name=f"u{img}")
                nc.vector.tensor_add(u[:], u0[:], ac[0])
                v0 = msk.tile([128, FD], f16, tag="u0", name=f"v0_{img}")
                nc.scalar.activation(v0[:], ac[2], Act.Copy, bias=0.0,
                                     scale=K2)
                v = msk.tile([128, FD], f16, tag="v", name=f"v{img}")
                nc.vector.tensor_add(v[:], v0[:], u[:])
                bright = msk.tile([128, FD], f16, tag="u", name=f"bright{img}")
                nc.scalar.activation(
                    bright[:], v[:], Act.Sigmoid, bias=s["b_br"], scale=br_s)
                s["bright"] = bright
                mx = msk.tile([128, FD], f16, tag="mx", name=f"mx{img}")
                nc.vector.tensor_max(mx[:], ac[0], ac[1])
                mx2 = msk.tile([128, FD], f16, tag="mx2", name=f"mx2{img}")
                nc.vector.tensor_max(mx2[:], mx[:], ac[2])
                mn = msk.tile([128, FD], f16, tag="mn", name=f"mn{img}")
                nc.vector.tensor_tensor(mn[:], ac[0], ac[1], Alu.min)
                mn2 = msk.tile([128, FD], f16, tag="mn2", name=f"mn2{img}")
                nc.vector.tensor_tensor(mn2[:], mn[:], ac[2], Alu.min)
                dsat = msk.tile([128, FD], f16, tag="mx", name=f"dsat{img}")
                nc.vector.tensor_sub(dsat[:], mx2[:], mn2[:])
                s["dsat"] = dsat

            def mask_back(img):
                s = st_img[img]
                lowsat = msk.tile([128, FD], f16, tag="mn",
                                  name=f"lowsat{img}")
                nc.scalar.activation(
                    lowsat[:], s["dsat"][:], Act.Sigmoid,
                    bias=s["b_ls"], scale=ls_s)
                mpad = msk.tile([128, HB * 528], f16, tag="mpad",
                                name=f"mpad{img}")
                mp3 = mpad[:].rearrange("p (b w) -> p b w", b=HB)
                nc.vector.memset(mp3[:, :, 0:8], 0.0)
                nc.vector.memset(mp3[:, :, 520:528], 0.0)
                nc.vector.tensor_mul(mp3[:, :, 8:520], s["bright"][:],
                                     lowsat[:])
                cs = msk.tile([128, HB * 528], f16, tag="csum",
                              name=f"cs{img}")
                nc.vector.tensor_tensor_scan(
                    cs[:], mpad[:], mpad[:], 0.0, Alu.add, Alu.bypass)
                c3 = cs[:].rearrange("p (b w) -> p b w", b=HB)
                pw = msk.tile([128, HB, W], f16, tag="pw", name=f"pw{img}")
                nc.vector.tensor_sub(pw[:], c3[:, :, 15:527],
                                     c3[:, :, 0:512])
                acc = ps.tile([128, HB, W], f32, tag="psum", name=f"acc{img}")
                s["acc"] = acc
                # warm up PE out of low p-state before the real matmuls
                nc.tensor.matmul(acc[:, 0, 0:100], s["kt"][:, 0, :],
                                 pw[:, 0, 0:100], start=True, stop=True)
                for hb in range(HB):
                    terms = [(1, hb)]
                    if hb > 0:
                        terms.append((2, hb - 1))
                    if hb < HB - 1:
                        terms.append((0, hb + 1))
                    for i, (mat, src) in enumerate(terms):
                        nc.tensor.matmul(
                            acc[:, hb, :],
                            s["kt"][:, {1: 0, 2: 1, 0: 2}[mat], :],
                            pw[:, src, :],
                            start=(i == 0),
                            stop=(i == len(terms) - 1),
                        )

            def chunk_a(img, hb):
                s = st_img[img]
                base = (img * HB + hb) * 6
                part = s["part"]
                sl = slice(hb * W, hb * W + W)
                p3 = s["p"][:].rearrange("p (c f) -> p c f", c=C)[:, :, sl]
                t3 = s["t"][:].rearrange("p (c f) -> p c f", c=C)[:, :, sl]
                ah = s["a"][:].rearrange("p (c f) -> p c f", c=C)[:, :, sl]

                d = wk.tile([128, C, W], f16, tag="d", name=f"d{img}_{hb}")
                nc.vector.tensor_sub(d[:], p3, t3)
                K = wk.tile([128, 4, C, W], f16, tag="K", name=f"K{img}_{hb}")
                nc.scalar.activation(
                    K[:, 0], d[:], Act.Abs,
                    accum_out=part[:, base + 1:base + 2])
                st = wk.tile([128, C, W], f16, tag="st", name=f"st{img}_{hb}")
                nc.vector.tensor_sub(st[:], t3, ah)
                sp = wk.tile([128, C, W], f16, tag="sp", name=f"sp{img}_{hb}")
                nc.vector.tensor_add(sp[:], st[:], d[:])
                nc.scalar.activation(K[:, 2], st[:], Act.Square)
                nc.scalar.activation(K[:, 3], sp[:], Act.Square)
                nc.vector.tensor_mul(K[:, 1], st[:], sp[:])
                SA = wk.tile([128, 4, W], f16, tag="SA", name=f"SA{img}_{hb}")
                nc.vector.tensor_add(SA[:], K[:, :, 0, :], K[:, :, 1, :])
                SB = wk.tile([128, 4, W], f16, tag="SB", name=f"SB{img}_{hb}")
                nc.vector.tensor_add(SB[:], SA[:], K[:, :, 2, :])
                gp = wk.tile([128, W], f16, tag="gp", name=f"gp{img}_{hb}")
                nc.vector.tensor_mul(gp[:], SB[:, 2, :], SB[:, 3, :])
                s[f"D{hb}"] = SB[:, 0, :]
                s[f"stsp{hb}"] = SB[:, 1, :]
                s[f"spsp{hb}"] = SB[:, 3, :]
                s[f"gp{hb}"] = gp

            def wm_rg(img):
                s = st_img[img]
                base = img * HB * 6
                wm = wk.tile([128, FD], f16, tag="wm", name=f"wm{img}")
                nc.scalar.activation(
                    wm[:], s["acc"][:].rearrange("p b w -> p (b w)"),
                    Act.Identity, accum_out=s["part"][:, base:base + 1])
                for hb in range(HB):
                    rg = wk.tile([128, W], f16, tag="rg", name=f"rg{img}_{hb}")
                    act_rsqrt(rg[:], s[f"gp{hb}"][:], s["b_eps"])
                    s[f"wm{hb}"] = wm[:, hb * W:(hb + 1) * W]
                    s[f"rg{hb}"] = rg[:]

            def tails(img):
                s = st_img[img]
                for hb in range(HB):
                    base = (img * HB + hb) * 6
                    part = s["part"]
                    wm, rg = s[f"wm{hb}"], s[f"rg{hb}"]
                    h = wk.tile([128, W], f16, tag="h", name=f"h{img}_{hb}")
                    nc.vector.tensor_mul(h[:], rg, wm)
                    q1 = wk.tile([128, W], f16, tag="q1",
                                 name=f"q1_{img}_{hb}")
                    nc.vector.tensor_mul(q1[:], s[f"spsp{hb}"], h[:])
                    q2 = wk.tile([128, W], f16, tag="q2",
                                 name=f"q2_{img}_{hb}")
                    nc.vector.tensor_sub(q2[:], q1[:], wm)
                    scr = wk.tile([128, W], f16, tag="scr",
                                  name=f"scr{img}_{hb}")
                    nc.scalar.activation(
                        scr[:], q2[:], Act.Abs,
                        accum_out=part[:, base + 2:base + 3])
                    q3 = wk.tile([128, W], f16, tag="q3",
                                 name=f"q3_{img}_{hb}")
                    nc.vector.tensor_mul(q3[:], s[f"stsp{hb}"], h[:])
                    nc.vector.tensor_scalar(
                        q3[:], q3[:], 1.0, None, Alu.mult, Alu.add,
                        accum_out=part[:, base + 3:base + 4])
                    Dw = wk.tile([128, W], f16, tag="Dw",
                                 name=f"Dw{img}_{hb}")
                    nc.vector.tensor_mul(Dw[:], s[f"D{hb}"], wm)
                    nc.vector.tensor_scalar(
                        Dw[:], Dw[:], 1.0, None, Alu.mult, Alu.add,
                        accum_out=part[:, base + 4:base + 5])

            # ---------- global emission order ----------
            dma_inputs(0, 0)
            kt = cst.tile([128, 3, 128], f16, tag="bands")
            for j in range(3):
                nc.sync.dma_start(kt[:, j, :], k_d[j])
            part = cst.tile([128, NSLOT], f32, tag="part")
            nc.vector.memset(part[:], 0.0)
            b_br = cst.tile([128, 1], f32, tag="b_br")
            nc.vector.memset(b_br[:], br_b)
            b_ls = cst.tile([128, 1], f32, tag="b_ls")
            nc.vector.memset(b_ls[:], ls_b)
            b_eps = cst.tile([128, 1], f32, tag="b_eps")
            nc.vector.memset(b_eps[:], EPS_G)
            for s in st_img:
                s["kt"] = kt
                s["part"] = part
                s["b_br"] = b_br[:]
                s["b_ls"] = b_ls[:]
                s["b_eps"] = b_eps[:]

            dma_inputs(0, 1)
            mask_front(0)
            chunk_a(0, 0)
            dma_inputs(1, 0)
            mask_back(0)
            mask_front(1)
            chunk_a(0, 1)
            mask_back(1)
            chunk_a(0, 2)
            chunk_a(0, 3)
            wm_rg(0)
            dma_inputs(1, 1)
            tails(0)
            chunk_a(1, 0)
            chunk_a(1, 1)
            chunk_a(1, 2)
            chunk_a(1, 3)
            wm_rg(1)
            tails(1)

            nc.sync.dma_start(o_d[:], part[:])

    nc.compile()
    return nc


def _get_nc(rescale):
    key = bool(rescale)
    if key not in _COMPILED:
        cs, cb = (0.5, 0.5) if rescale else (1.0, 0.0)
        _COMPILED[key] = _build(
            20.0 * 0.299 * cs, 20.0 * (cb * (0.299 + 0.587 + 0.114) - 0.65),
            -20.0 * cs, 20.0 * 0.15,
        )
    return _COMPILED[key]


def _host_layout(x16):
    # [B,C,H,W] fp16 -> per-core [BPC,128,SLAB]; slab free = (c, hb, w)
    xs = x16.reshape(NCORES, BPC, C, HB, 128, W).transpose(0, 1, 4, 2, 3, 5)
    return np.ascontiguousarray(xs.reshape(NCORES, BPC, 128, SLAB))


def kernel(pred, target, source, _trace=False):
    from concourse.bass_utils import run_bass_kernel_spmd

    rescale = bool(source.min() < 0)
    nc = _get_nc(rescale)

    p = _host_layout(pred.astype(np.float16))
    t = _host_layout(target.astype(np.float16))
    a = _host_layout(source.astype(np.float16))
    k = _band_matrices()

    in_maps = [
        {"p": p[i], "t": t[i], "a": a[i], "k": k} for i in range(NCORES)
    ]
    res = run_bass_kernel_spmd(
        nc, in_maps, core_ids=list(range(NCORES)), trace=_trace
    )
    parts = np.stack([r["o"] for r in res.results])  # [8,128,NSLOT]
    ps = parts.sum(axis=(0, 1), dtype=np.float64)    # [NSLOT]
    sl = ps[:BPC * HB * 6].reshape(BPC * HB, 6)
    s_m, sd, s_yw, s_ww, s_dw = (sl[:, j].sum() for j in range(5))
    total = (4.0 / (3 * N_TOT)) * sd + (2.0 / (225.0 * N_TOT)) * (
        s_m + 2.0 * s_dw + 0.5 * s_yw - s_ww
    )
    out = np.float32(total)
    if _trace:
        return out, res
    return out


# revision 20
# speedup vs baseline: 1.3293x; 1.0024x over previous
"""Trainium2 Bass kernel for ComprehensiveWindowAwareLoss (v5).

Self-contained: hardcodes shapes [16,3,512,512] f32, shards batch across 8
NeuronCores (2 images/core). Host converts inputs to fp16 and lays each
image out as a [128, 6144] slab: free = (c, hb, w), h = hb*128 + p.

Engine plan (HW-validated ops only; Pool/GpSimd has no elementwise support
in this compile pipeline):
  DVE:  mask chain (u/v brightness combine, max3/min3/dsat, mprod,
        W-pool cumsum + shifted subtract) and the color/L1 chunks
        (d, st, sp, q=st*sp, channel sums, gp, wm tails + ts-accums)
  Act:  sigmoids, e=|d| (Abs, accum -> SD), st^2/sp^2 squares,
        |spsp*h-wm| (Abs, accum -> S_yw), per-hb wm copies (accum S_m),
        rsqrt(stst*spsp+eps) via direct InstActivation
  PE:   H-pool band matmuls (one PSUM bank per hb)
Host: total = 4/(3N)*SD + 2/(225N)*(S_m + 2*S_Dw + 0.5*S_yw - S_ww).
Source rescale ((x+1)/2 when min<0) is folded into sigmoid constants.
Emission order is hand-scheduled (engines execute in-order).
"""
import numpy as np

B, C, H, W = 16, 3, 512, 512
NCORES = 8
BPC = B // NCORES      # images per core
HB = H // 128          # 4 h-blocks
FD = HB * W            # 2048 free elems per channel map
SLAB = C * FD          # 6144 free elems per slab
K1 = 0.587 / 0.299
K2 = 0.114 / 0.299
N_TOT = B * H * W
EPS_G = 4e-3           # eps inside rsqrt(stst*spsp + eps); keeps fp16 safe
NSLOT = 48             # accumulator slots: (img, hb) x 6

_COMPILED = {}


def _band_matrices():
    k = np.arange(128)[:, None]
    m = np.arange(128)[None, :]
    B_mid = (np.abs(k - m) <= 7).astype(np.float16)
    B_up = ((k - m >= 121) & (k - m <= 135)).astype(np.float16)
    B_dn = ((m - k >= 121) & (m - k <= 135)).astype(np.float16)
    return np.stack([B_mid, B_up, B_dn])  # [3,128,128]


def _build(br_s, br_b, ls_s, ls_b):
    import concourse.bass as bass
    import concourse.bacc as bacc
    import concourse.tile as tile
    from concourse import mybir

    f16 = mybir.dt.float16
    f32 = mybir.dt.float32
    Alu = mybir.AluOpType
    Act = mybir.ActivationFunctionType

    nc = bacc.Bacc("TRN2", debug=False, num_devices=NCORES)

    def act_rsqrt(out, in_, bias_ap):
        # nc.scalar.activation refuses Rsqrt (HW accuracy ~3e-4 measured;
        # fine at our tolerance); emit the InstActivation directly.
        eng = nc.scalar
        return eng.add_instruction(
            mybir.InstActivation(
                name=nc.get_next_instruction_name(),
                func=Act.Rsqrt,
                ins=[
                    eng.lower_ap(in_),
                    eng.lower_ap(bias_ap),
                    mybir.ImmediateValue(dtype=f32, value=1.0),
                    mybir.ImmediateValue(dtype=f32, value=0.0),
                ],
                outs=[eng.lower_ap(out)],
            )
        )

    p_d = nc.dram_tensor("p", [BPC, 128, SLAB], f16, kind="ExternalInput").ap()
    t_d = nc.dram_tensor("t", [BPC, 128, SLAB], f16, kind="ExternalInput").ap()
    a_d = nc.dram_tensor("a", [BPC, 128, SLAB], f16, kind="ExternalInput").ap()
    k_d = nc.dram_tensor("k", [3, 128, 128], f16, kind="ExternalInput").ap()
    o_d = nc.dram_tensor("o", [128, NSLOT], f32, kind="ExternalOutput").ap()

    with tile.TileContext(nc) as tc:
        with (
            tc.tile_pool(name="inp", bufs=2) as inp,
            tc.tile_pool(name="msk", bufs=1) as msk,
            tc.tile_pool(name="wk", bufs=2) as wk,
            tc.tile_pool(name="cst", bufs=1) as cst,
            tc.tile_pool(name="ps", bufs=2, space=bass.MemorySpace.PSUM) as ps,
        ):
            st_img = [{} for _ in range(BPC)]

            def dma_inputs(img, phase):
                s = st_img[img]
                if phase == 0:
                    a = inp.tile([128, SLAB], f16, tag="a", name=f"a{img}")
                    s["a"] = a
                    nc.sync.dma_start(a[:, 0:2 * FD], a_d[img][:, 0:2 * FD])
                    p = inp.tile([128, SLAB], f16, tag="p", name=f"p{img}")
                    t = inp.tile([128, SLAB], f16, tag="t", name=f"t{img}")
                    s["p"], s["t"] = p, t
                    pr = p_d[img].rearrange("p (c b f) -> p c b f", c=C, b=HB)
                    tr = t_d[img].rearrange("p (c b f) -> p c b f", c=C, b=HB)
                    p3 = p[:].rearrange("p (c b f) -> p c b f", c=C, b=HB)
                    t3 = t[:].rearrange("p (c b f) -> p c b f", c=C, b=HB)
                    s["pr"], s["tr"], s["p3"], s["t3"] = pr, tr, p3, t3
                    nc.sync.dma_start(a[:, 2 * FD:3 * FD],
                                      a_d[img][:, 2 * FD:3 * FD])
                    nc.sync.dma_start(p3[:, :, 0, :], pr[:, :, 0, :])
                    nc.sync.dma_start(t3[:, :, 0, :], tr[:, :, 0, :])
                else:
                    for hb in range(1, HB):
                        nc.sync.dma_start(
                            s["p3"][:, :, hb, :], s["pr"][:, :, hb, :])
                        nc.sync.dma_start(
                            s["t3"][:, :, hb, :], s["tr"][:, :, hb, :])

            def mask_front(img):
                s = st_img[img]
                a = s["a"]
                ac = [a[:, c * FD:(c + 1) * FD] for c in range(C)]
                u0 = msk.tile([128, FD], f16, tag="u0", name=f"u0_{img}")
                nc.scalar.activation(u0[:], ac[1], Act.Copy, bias=0.0,
                                     scale=K1)
                u = msk.tile([128, FD], f16, tag="u", name=f"u{img}")
                nc.vector.tensor_add(u[:], u0[:], ac[0])
                v0 = msk.tile([128, FD], f16, tag="u0", name=f"v0_{img}")
                nc.scalar.activation(v0[:], ac[2], Act.Copy, bias=0.0,
                                     scale=K2)
                v = msk.tile([128, FD], f16, tag="v", name=f"v{img}")
                nc.vector.tensor_add(v[:], v0[:], u[:])
                bright = msk.tile([128, FD], f16, tag="u", name=f"bright{img}")
                nc.scalar.activation(
                    bright[:], v[:], Act.Sigmoid, bias=s["b_br"], scale=br_s)
                s["bright"] = bright
                mx = msk.tile([128, FD], f16, tag="mx", name=f"mx{img}")
                nc.vector.tensor_max(mx[:], ac[0], ac[1])
                mx2 = msk.tile([128, FD], f16, tag="mx2", name=f"mx2{img}")
                nc.vector.tensor_max(mx2[:], mx[:], ac[2])
                mn = msk.tile([128, FD], f16, tag="mn", name=f"mn{img}")
                nc.vector.tensor_tensor(mn[:], ac[0], ac[1], Alu.min)
                mn2 = msk.tile([128, FD], f16, tag="mn2", name=f"mn2{img}")
                nc.vector.tensor_tensor(mn2[:], mn[:], ac[2], Alu.min)
                dsat = msk.tile([128, FD], f16, tag="mx", name=f"dsat{img}")
                nc.vector.tensor_sub(dsat[:], mx2[:], mn2[:])
                s["dsat"] = dsat

            def mask_back(img):
                s = st_img[img]
                lowsat = msk.tile([128, FD], f16, tag="mn",
                                  name=f"lowsat{img}")
                nc.scalar.activation(
                    lowsat[:], s["dsat"][:], Act.Sigmoid,
                    bias=s["b_ls"], scale=ls_s)
                mpad = msk.tile([128, HB * 528], f16, tag="mpad",
                                name=f"mpad{img}")
                mp3 = mpad[:].rearrange("p (b w) -> p b w", b=HB)
                nc.vector.memset(mp3[:, :, 0:8], 0.0)
                nc.vector.memset(mp3[:, :, 520:528], 0.0)
                nc.vector.tensor_mul(mp3[:, :, 8:520], s["bright"][:],
                                     lowsat[:])
                cs = msk.tile([128, HB * 528], f16, tag="csum",
                              name=f"cs{img}")
                nc.vector.tensor_tensor_scan(
                    cs[:], mpad[:], mpad[:], 0.0, Alu.add, Alu.bypass)
                c3 = cs[:].rearrange("p (b w) -> p b w", b=HB)
                pw = msk.tile([128, HB, W], f16, tag="pw", name=f"pw{img}")
                nc.vector.tensor_sub(pw[:], c3[:, :, 15:527],
                                     c3[:, :, 0:512])
                acc = ps.tile([128, HB, W], f32, tag="psum", name=f"acc{img}")
                s["acc"] = acc
                # warm up PE out of low p-state before the real matmuls
                nc.tensor.matmul(acc[:, 0, 0:100], s["kt"][:, 0, :],
                                 pw[:, 0, 0:100], start=True, stop=True)
                for hb in range(HB):
                    terms = [(1, hb)]
                    if hb > 0:
                        terms.append((2, hb - 1))
                    if hb < HB - 1:
                        terms.append((0, hb + 1))
                    for i, (mat, src) in enumerate(terms):
                        nc.tensor.matmul(
                            acc[:, hb, :],
                            s["kt"][:, {1: 0, 2: 1, 0: 2}[mat], :],
                            pw[:, src, :],
                            start=(i == 0),
                            stop=(i == len(terms) - 1),
                        )

            def chunk_a(img, hb):
                s = st_img[img]
                base = (img * HB + hb) * 6
                part = s["part"]
                sl = slice(hb * W, hb * W + W)
                p3 = s["p"][:].rearrange("p (c f) -> p c f", c=C)[:, :, sl]
                t3 = s["t"][:].rearrange("p (c f) -> p c f", c=C)[:, :, sl]
                ah = s["a"][:].rearrange("p (c f) -> p c f", c=C)[:, :, sl]

                d = wk.tile([128, C, W], f16, tag="d", name=f"d{img}_{hb}")
                nc.vector.tensor_sub(d[:], p3, t3)
                K = wk.tile([128, 4, C, W], f16, tag="K", name=f"K{img}_{hb}")
                nc.scalar.activation(
                    K[:, 0], d[:], Act.Abs,
                    accum_out=part[:, base + 1:base + 2])
                st = wk.tile([128, C, W], f16, tag="st", name=f"st{img}_{hb}")
                nc.vector.tensor_sub(st[:], t3, ah)
                sp = wk.tile([128, C, W], f16, tag="sp", name=f"sp{img}_{hb}")
                nc.vector.tensor_add(sp[:], st[:], d[:])
                nc.scalar.activation(K[:, 2], st[:], Act.Square)
                nc.scalar.activation(K[:, 3], sp[:], Act.Square)
                nc.vector.tensor_mul(K[:, 1], st[:], sp[:])
                SA = wk.tile([128, 4, W], f16, tag="SA", name=f"SA{img}_{hb}")
                nc.vector.tensor_add(SA[:], K[:, :, 0, :], K[:, :, 1, :])
                SB = wk.tile([128, 4, W], f16, tag="SB", name=f"SB{img}_{hb}")
                nc.vector.tensor_add(SB[:], SA[:], K[:, :, 2, :])
                gp = wk.tile([128, W], f16, tag="gp", name=f"gp{img}_{hb}")
                nc.vector.tensor_mul(gp[:], SB[:, 2, :], SB[:, 3, :])
                s[f"D{hb}"] = SB[:, 0, :]
                s[f"stsp{hb}"] = SB[:, 1, :]
                s[f"spsp{hb}"] = SB[:, 3, :]
                s[f"gp{hb}"] = gp

            def wm_rg(img):
                s = st_img[img]
                base = img * HB * 6
                wm = wk.tile([128, FD], f16, tag="wm", name=f"wm{img}")
                nc.scalar.activation(
                    wm[:], s["acc"][:].rearrange("p b w -> p (b w)"),
                    Act.Identity, accum_out=s["part"][:, base:base + 1])
                for hb in range(HB):
                    rg = wk.tile([128, W], f16, tag="rg", name=f"rg{img}_{hb}")
                    act_rsqrt(rg[:], s[f"gp{hb}"][:], s["b_eps"])
                    s[f"wm{hb}"] = wm[:, hb * W:(hb + 1) * W]
                    s[f"rg{hb}"] = rg[:]

            def tails(img):
                s = st_img[img]
                for hb in range(HB):
                    base = (img * HB + hb) * 6
                    part = s["part"]
                    wm, rg = s[f"wm{hb}"], s[f"rg{hb}"]
                    h = wk.tile([128, W], f16, tag="h", name=f"h{img}_{hb}")
                    nc.vector.tensor_mul(h[:], rg, wm)
                    q1 = wk.tile([128, W], f16, tag="q1",
                                 name=f"q1_{img}_{hb}")
                    nc.vector.tensor_mul(q1[:], s[f"spsp{hb}"], h[:])
                    q2 = wk.tile([128, W], f16, tag="q2",
                                 name=f"q2_{img}_{hb}")
                    nc.vector.tensor_sub(q2[:], q1[:], wm)
                    scr = wk.tile([128, W], f16, tag="scr",
                                  name=f"scr{img}_{hb}")
                    nc.scalar.activation(
                        scr[:], q2[:], Act.Abs,
                        accum_out=part[:, base + 2:base + 3])
                    q3 = wk.tile([128, W], f16, tag="q3",
                                 name=f"q3_{img}_{hb}")
                    nc.vector.tensor_mul(q3[:], s[f"stsp{hb}"], h[:])
                    nc.vector.tensor_scalar(
                        q3[:], q3[:], 1.0, None, Alu.mult, Alu.add,
                        accum_out=part[:, base + 3:base + 4])
                    Dw = wk.tile([128, W], f16, tag="Dw",
                                 name=f"Dw{img}_{hb}")
                    nc.vector.tensor_mul(Dw[:], s[f"D{hb}"], wm)
                    nc.vector.tensor_scalar(
                        Dw[:], Dw[:], 1.0, None, Alu.mult, Alu.add,
                        accum_out=part[:, base + 4:base + 5])

            # ---------- global emission order ----------
            dma_inputs(0, 0)
            kt = cst.tile([128, 3, 128], f16, tag="bands")
            for j in range(3):
                nc.sync.dma_start(kt[:, j, :], k_d[j])
            part = cst.tile([128, NSLOT], f32, tag="part")
            nc.vector.memset(part[:], 0.0)
            b_br = cst.tile([128, 1], f32, tag="b_br")
            nc.vector.memset(b_br[:], br_b)
            b_ls = cst.tile([128, 1], f32, tag="b_ls")
            nc.vector.memset(b_ls[:], ls_b)
            b_eps = cst.tile([128, 1], f32, tag="b_eps")
            nc.vector.memset(b_eps[:], EPS_G)
            for s in st_img:
                s["kt"] = kt
                s["part"] = part
                s["b_br"] = b_br[:]
                s["b_ls"] = b_ls[:]
                s["b_eps"] = b_eps[:]

            dma_inputs(0, 1)
            mask_front(0)
            chunk_a(0, 0)
            dma_inputs(1, 0)
            mask_back(0)
            mask_front(1)
            chunk_a(0, 1)
            mask_back(1)
            chunk_a(0, 2)
            chunk_a(0, 3)
            wm_rg(0)
            dma_inputs(1, 1)
            tails(0)
            chunk_a(1, 0)
            chunk_a(1, 1)
            chunk_a(1, 2)
            chunk_a(1, 3)
            wm_rg(1)
            tails(1)

            nc.sync.dma_start(o_d[:], part[:])

    nc.compile()
    return nc


def _get_nc(rescale):
    key = bool(rescale)
    if key not in _COMPILED:
        cs, cb = (0.5, 0.5) if rescale else (1.0, 0.0)
        _COMPILED[key] = _build(
            20.0 * 0.299 * cs, 20.0 * (cb * (0.299 + 0.587 + 0.114) - 0.65),
            -20.0 * cs, 20.0 * 0.15,
        )
    return _COMPILED[key]


def _host_layout(x16):
    # [B,C,H,W] fp16 -> per-core [BPC,128,SLAB]; slab free = (c, hb, w)
    xs = x16.reshape(NCORES, BPC, C, HB, 128, W).transpose(0, 1, 4, 2, 3, 5)
    return np.ascontiguousarray(xs.reshape(NCORES, BPC, 128, SLAB))


def kernel(pred, target, source, _trace=False):
    from concourse.bass_utils import run_bass_kernel_spmd

    rescale = bool(source.min() < 0)
    nc = _get_nc(rescale)

    p = _host_layout(pred.astype(np.float16))
    t = _host_layout(target.astype(np.float16))
    a = _host_layout(source.astype(np.float16))
    k = _band_matrices()

    in_maps = [
        {"p": p[i], "t": t[i], "a": a[i], "k": k} for i in range(NCORES)
    ]
    res = run_bass_kernel_spmd(
        nc, in_maps, core_ids=list(range(NCORES)), trace=_trace
    )
    parts = np.stack([r["o"] for r in res.results])  # [8,128,NSLOT]
    ps = parts.sum(axis=(0, 1), dtype=np.float64)    # [NSLOT]
    sl = ps[:BPC * HB * 6].reshape(BPC * HB, 6)
    s_m, sd, s_yw, s_ww, s_dw = (sl[:, j].sum() for j in range(5))
    total = (4.0 / (3 * N_TOT)) * sd + (2.0 / (225.0 * N_TOT)) * (
        s_m + 2.0 * s_dw + 0.5 * s_yw - s_ww
    )
    out = np.float32(total)
    if _trace:
        return out, res
    return out


# revision 30
# speedup vs baseline: 1.3696x; 1.0303x over previous
"""Trainium2 Bass kernel for ComprehensiveWindowAwareLoss (v5).

Self-contained: hardcodes shapes [16,3,512,512] f32, shards batch across 8
NeuronCores (2 images/core). Host converts inputs to fp16 and lays each
image out as a [128, 6144] slab: free = (c, hb, w), h = hb*128 + p.

Engine plan (HW-validated ops only; Pool/GpSimd has no elementwise support
in this compile pipeline):
  DVE:  mask chain (u/v brightness combine, max3/min3/dsat, mprod,
        W-pool cumsum + shifted subtract) and the color/L1 chunks
        (d, st, sp, q=st*sp, channel sums, gp, wm tails + ts-accums)
  Act:  sigmoids, K1/K2 channel scales, e=|d| (Abs, accum -> SD),
        st^2/sp^2 squares, |spsp*h-wm| (Abs, accum -> S_yw), whole-image
        wm copy (PSUM->SBUF, accum S_m), rsqrt(stst*spsp+eps) via direct
        InstActivation
  PE:   H-pool band matmuls (one PSUM bank per hb, with p-state warmup)
The per-chunk |d|/st*sp/st^2/sp^2 maps are packed into one K[128,4,3,512]
tile so all four channel sums reduce in two wide strided adds (SA/SB);
D/stsp/stst/spsp live as SB lanes.
Host: total = 4/(3N)*SD + 2/(225N)*(S_m + 2*S_Dw + 0.5*S_yw - S_ww).
Source rescale ((x+1)/2 when min<0) is folded into sigmoid constants.
Emission order is hand-scheduled (engines execute in-order).
"""
import numpy as np

B, C, H, W = 16, 3, 512, 512
NCORES = 8
BPC = B // NCORES      # images per core
HB = H // 128          # 4 h-blocks
FD = HB * W            # 2048 free elems per channel map
SLAB = C * FD          # 6144 free elems per slab
K1 = 0.587 / 0.299
K2 = 0.114 / 0.299
N_TOT = B * H * W
EPS_G = 4e-3           # eps inside rsqrt(stst*spsp + eps); keeps fp16 safe
NSLOT = 48             # accumulator slots: (img, hb) x 6

_COMPILED = {}


def _band_matrices():
    k = np.arange(128)[:, None]
    m = np.arange(128)[None, :]
    B_mid = (np.abs(k - m) <= 7).astype(np.float16)
    B_up = ((k - m >= 121) & (k - m <= 135)).astype(np.float16)
    B_dn = ((m - k >= 121) & (m - k <= 135)).astype(np.float16)
    return np.stack([B_mid, B_up, B_dn])  # [3,128,128]


def _build(br_s, br_b, ls_s, ls_b):
    import concourse.bass as bass
    import concourse.bacc as bacc
    import concourse.tile as tile
    from concourse import mybir

    f16 = mybir.dt.float16
    f32 = mybir.dt.float32
    Alu = mybir.AluOpType
    Act = mybir.ActivationFunctionType

    nc = bacc.Bacc("TRN2", debug=False, num_devices=NCORES)

    def act_rsqrt(out, in_, bias_ap):
        # nc.scalar.activation refuses Rsqrt (HW accuracy ~3e-4 measured;
        # fine at our tolerance); emit the InstActivation directly.
        eng = nc.scalar
        return eng.add_instruction(
            mybir.InstActivation(
                name=nc.get_next_instruction_name(),
                func=Act.Rsqrt,
                ins=[
                    eng.lower_ap(in_),
                    eng.lower_ap(bias_ap),
                    mybir.ImmediateValue(dtype=f32, value=1.0),
                    mybir.ImmediateValue(dtype=f32, value=0.0),
                ],
                outs=[eng.lower_ap(out)],
            )
        )

    p_d = nc.dram_tensor("p", [BPC, 128, SLAB], f16, kind="ExternalInput").ap()
    t_d = nc.dram_tensor("t", [BPC, 128, SLAB], f16, kind="ExternalInput").ap()
    a_d = nc.dram_tensor("a", [BPC, 128, SLAB], f16, kind="ExternalInput").ap()
    k_d = nc.dram_tensor("k", [3, 128, 128], f16, kind="ExternalInput").ap()
    o_d = nc.dram_tensor("o", [128, NSLOT], f32, kind="ExternalOutput").ap()

    with tile.TileContext(nc) as tc:
        with (
            tc.tile_pool(name="inp", bufs=2) as inp,
            tc.tile_pool(name="msk", bufs=1) as msk,
            tc.tile_pool(name="wk", bufs=2) as wk,
            tc.tile_pool(name="cst", bufs=1) as cst,
            tc.tile_pool(name="ps", bufs=2, space=bass.MemorySpace.PSUM) as ps,
        ):
            st_img = [{} for _ in range(BPC)]

            def dma_inputs(img, phase):
                s = st_img[img]
                if phase == 0:
                    a = inp.tile([128, SLAB], f16, tag="a", name=f"a{img}")
                    s["a"] = a
                    nc.sync.dma_start(a[:, 0:2 * FD], a_d[img][:, 0:2 * FD])
                    p = inp.tile([128, SLAB], f16, tag="p", name=f"p{img}")
                    t = inp.tile([128, SLAB], f16, tag="t", name=f"t{img}")
                    s["p"], s["t"] = p, t
                    pr = p_d[img].rearrange("p (c b f) -> p c b f", c=C, b=HB)
                    tr = t_d[img].rearrange("p (c b f) -> p c b f", c=C, b=HB)
                    p3 = p[:].rearrange("p (c b f) -> p c b f", c=C, b=HB)
                    t3 = t[:].rearrange("p (c b f) -> p c b f", c=C, b=HB)
                    s["pr"], s["tr"], s["p3"], s["t3"] = pr, tr, p3, t3
                    nc.sync.dma_start(a[:, 2 * FD:3 * FD],
                                      a_d[img][:, 2 * FD:3 * FD])
                    nc.sync.dma_start(p3[:, :, 0, :], pr[:, :, 0, :])
                    nc.sync.dma_start(t3[:, :, 0, :], tr[:, :, 0, :])
                else:
                    for hb in range(1, HB):
                        nc.sync.dma_start(
                            s["p3"][:, :, hb, :], s["pr"][:, :, hb, :])
                        nc.sync.dma_start(
                            s["t3"][:, :, hb, :], s["tr"][:, :, hb, :])

            def mask_front(img):
                s = st_img[img]
                a = s["a"]
                ac = [a[:, c * FD:(c + 1) * FD] for c in range(C)]
                u0 = msk.tile([128, FD], f16, tag="u0", name=f"u0_{img}")
                nc.scalar.activation(u0[:], ac[1], Act.Copy, bias=0.0,
                                     scale=K1)
                u = msk.tile([128, FD], f16, tag="u", name=f"u{img}")
                nc.vector.tensor_add(u[:], u0[:], ac[0])
                v0 = msk.tile([128, FD], f16, tag="u0", name=f"v0_{img}")
                nc.scalar.activation(v0[:], ac[2], Act.Copy, bias=0.0,
                                     scale=K2)
                v = msk.tile([128, FD], f16, tag="v", name=f"v{img}")
                nc.vector.tensor_add(v[:], v0[:], u[:])
                bright = msk.tile([128, FD], f16, tag="u", name=f"bright{img}")
                nc.scalar.activation(
                    bright[:], v[:], Act.Sigmoid, bias=s["b_br"], scale=br_s)
                s["bright"] = bright
                mx = msk.tile([128, FD], f16, tag="mx", name=f"mx{img}")
                nc.vector.tensor_max(mx[:], ac[0], ac[1])
                mx2 = msk.tile([128, FD], f16, tag="mx2", name=f"mx2{img}")
                nc.vector.tensor_max(mx2[:], mx[:], ac[2])
                mn = msk.tile([128, FD], f16, tag="mn", name=f"mn{img}")
                nc.vector.tensor_tensor(mn[:], ac[0], ac[1], Alu.min)
                mn2 = msk.tile([128, FD], f16, tag="mn2", name=f"mn2{img}")
                nc.vector.tensor_tensor(mn2[:], mn[:], ac[2], Alu.min)
                dsat = msk.tile([128, FD], f16, tag="mx", name=f"dsat{img}")
                nc.vector.tensor_sub(dsat[:], mx2[:], mn2[:])
                s["dsat"] = dsat

            def mask_back(img):
                s = st_img[img]
                lowsat = msk.tile([128, FD], f16, tag="mn",
                                  name=f"lowsat{img}")
                nc.scalar.activation(
                    lowsat[:], s["dsat"][:], Act.Sigmoid,
                    bias=s["b_ls"], scale=ls_s)
                mpad = msk.tile([128, HB * 528], f16, tag="mpad",
                                name=f"mpad{img}")
                mp3 = mpad[:].rearrange("p (b w) -> p b w", b=HB)
                nc.vector.memset(mp3[:, :, 0:8], 0.0)
                nc.vector.memset(mp3[:, :, 520:528], 0.0)
                nc.vector.tensor_mul(mp3[:, :, 8:520], s["bright"][:],
                                     lowsat[:])
                cs = msk.tile([128, HB * 528], f16, tag="csum",
                              name=f"cs{img}")
                nc.vector.tensor_tensor_scan(
                    cs[:], mpad[:], mpad[:], 0.0, Alu.add, Alu.bypass)
                c3 = cs[:].rearrange("p (b w) -> p b w", b=HB)
                pw = msk.tile([128, HB, W], f16, tag="pw", name=f"pw{img}")
                nc.vector.tensor_sub(pw[:], c3[:, :, 15:527],
                                     c3[:, :, 0:512])
                acc = ps.tile([128, HB, W], f32, tag="psum", name=f"acc{img}")
                s["acc"] = acc
                # warm up PE out of low p-state before the real matmuls
                nc.tensor.matmul(acc[:, 0, 0:100], s["kt"][:, 0, :],
                                 pw[:, 0, 0:100], start=True, stop=True)
                for hb in range(HB):
                    terms = [(1, hb)]
                    if hb > 0:
                        terms.append((2, hb - 1))
                    if hb < HB - 1:
                        terms.append((0, hb + 1))
                    for i, (mat, src) in enumerate(terms):
                        nc.tensor.matmul(
                            acc[:, hb, :],
                            s["kt"][:, {1: 0, 2: 1, 0: 2}[mat], :],
                            pw[:, src, :],
                            start=(i == 0),
                            stop=(i == len(terms) - 1),
                        )

            def chunk_a(img, hb):
                s = st_img[img]
                base = (img * HB + hb) * 6
                part = s["part"]
                sl = slice(hb * W, hb * W + W)
                p3 = s["p"][:].rearrange("p (c f) -> p c f", c=C)[:, :, sl]
                t3 = s["t"][:].rearrange("p (c f) -> p c f", c=C)[:, :, sl]
                ah = s["a"][:].rearrange("p (c f) -> p c f", c=C)[:, :, sl]

                d = wk.tile([128, C, W], f16, tag="d", name=f"d{img}_{hb}")
                nc.vector.tensor_sub(d[:], p3, t3)
                K = wk.tile([128, 4, C, W], f16, tag="K", name=f"K{img}_{hb}")
                nc.scalar.activation(
                    K[:, 0], d[:], Act.Abs,
                    accum_out=part[:, base + 1:base + 2])
                st = wk.tile([128, C, W], f16, tag="st", name=f"st{img}_{hb}")
                nc.vector.tensor_sub(st[:], t3, ah)
                sp = wk.tile([128, C, W], f16, tag="sp", name=f"sp{img}_{hb}")
                nc.vector.tensor_add(sp[:], st[:], d[:])
                nc.scalar.activation(K[:, 2], st[:], Act.Square)
                nc.scalar.activation(K[:, 3], sp[:], Act.Square)
                nc.vector.tensor_mul(K[:, 1], st[:], sp[:])
                SA = wk.tile([128, 4, W], f16, tag="SA", name=f"SA{img}_{hb}")
                nc.vector.tensor_add(SA[:], K[:, :, 0, :], K[:, :, 1, :])
                SB = wk.tile([128, 4, W], f16, tag="SB", name=f"SB{img}_{hb}")
                nc.vector.tensor_add(SB[:], SA[:], K[:, :, 2, :])
                gp = wk.tile([128, W], f16, tag="gp", name=f"gp{img}_{hb}")
                nc.vector.tensor_mul(gp[:], SB[:, 2, :], SB[:, 3, :])
                s[f"D{hb}"] = SB[:, 0, :]
                s[f"stsp{hb}"] = SB[:, 1, :]
                s[f"spsp{hb}"] = SB[:, 3, :]
                s[f"gp{hb}"] = gp

            def wm_copy(img):
                s = st_img[img]
                base = img * HB * 6
                wm = wk.tile([128, FD], f16, tag="wm", name=f"wm{img}")
                nc.scalar.activation(
                    wm[:], s["acc"][:].rearrange("p b w -> p (b w)"),
                    Act.Identity, accum_out=s["part"][:, base:base + 1])
                for hb in range(HB):
                    s[f"wm{hb}"] = wm[:, hb * W:(hb + 1) * W]

            def rgs(img, hbs):
                s = st_img[img]
                for hb in hbs:
                    rg = wk.tile([128, W], f16, tag="rg", name=f"rg{img}_{hb}")
                    act_rsqrt(rg[:], s[f"gp{hb}"][:], s["b_eps"])
                    s[f"rg{hb}"] = rg[:]

            def tails(img):
                s = st_img[img]
                for hb in range(HB):
                    base = (img * HB + hb) * 6
                    part = s["part"]
                    wm, rg = s[f"wm{hb}"], s[f"rg{hb}"]
                    h = wk.tile([128, W], f16, tag="h", name=f"h{img}_{hb}")
                    nc.vector.tensor_mul(h[:], rg, wm)
                    q1 = wk.tile([128, W], f16, tag="q1",
                                 name=f"q1_{img}_{hb}")
                    nc.vector.tensor_mul(q1[:], s[f"spsp{hb}"], h[:])
                    q2 = wk.tile([128, W], f16, tag="q2",
                                 name=f"q2_{img}_{hb}")
                    nc.vector.tensor_sub(q2[:], q1[:], wm)
                    scr = wk.tile([128, W], f16, tag="scr",
                                  name=f"scr{img}_{hb}")
                    nc.scalar.activation(
                        scr[:], q2[:], Act.Abs,
                        accum_out=part[:, base + 2:base + 3])
                    q3 = wk.tile([128, W], f16, tag="q3",
                                 name=f"q3_{img}_{hb}")
                    nc.vector.tensor_mul(q3[:], s[f"stsp{hb}"], h[:])
                    nc.vector.tensor_scalar(
                        q3[:], q3[:], 1.0, None, Alu.mult, Alu.add,
                        accum_out=part[:, base + 3:base + 4])
                    Dw = wk.tile([128, W], f16, tag="Dw",
                                 name=f"Dw{img}_{hb}")
                    nc.vector.tensor_mul(Dw[:], s[f"D{hb}"], wm)
                    nc.vector.tensor_scalar(
                        Dw[:], Dw[:], 1.0, None, Alu.mult, Alu.add,
                        accum_out=part[:, base + 4:base + 5])

            # ---------- global emission order ----------
            dma_inputs(0, 0)
            kt = cst.tile([128, 3, 128], f16, tag="bands")
            for j in range(3):
                nc.sync.dma_start(kt[:, j, :], k_d[j])
            part = cst.tile([128, NSLOT], f32, tag="part")
            nc.vector.memset(part[:], 0.0)
            b_br = cst.tile([128, 1], f32, tag="b_br")
            nc.vector.memset(b_br[:], br_b)
            b_ls = cst.tile([128, 1], f32, tag="b_ls")
            nc.vector.memset(b_ls[:], ls_b)
            b_eps = cst.tile([128, 1], f32, tag="b_eps")
            nc.vector.memset(b_eps[:], EPS_G)
            for s in st_img:
                s["kt"] = kt
                s["part"] = part
                s["b_br"] = b_br[:]
                s["b_ls"] = b_ls[:]
                s["b_eps"] = b_eps[:]

            dma_inputs(0, 1)
            mask_front(0)
            dma_inputs(1, 0)
            chunk_a(0, 0)
            mask_front(1)
            mask_back(0)
            chunk_a(0, 1)
            mask_back(1)
            chunk_a(0, 2)
            rgs(0, [0, 1, 2])
            chunk_a(0, 3)
            wm_copy(0)
            rgs(0, [3])
            dma_inputs(1, 1)
            chunk_a(1, 0)
            tails(0)
            chunk_a(1, 1)
            chunk_a(1, 2)
            rgs(1, [0, 1])
            chunk_a(1, 3)
            wm_copy(1)
            rgs(1, [2, 3])
            tails(1)

            nc.sync.dma_start(o_d[:], part[:])

    nc.compile()
    return nc


def _get_nc(rescale):
    key = bool(rescale)
    if key not in _COMPILED:
        cs, cb = (0.5, 0.5) if rescale else (1.0, 0.0)
        _COMPILED[key] = _build(
            20.0 * 0.299 * cs, 20.0 * (cb * (0.299 + 0.587 + 0.114) - 0.65),
            -20.0 * cs, 20.0 * 0.15,
        )
    return _COMPILED[key]


def _host_layout(x16):
    # [B,C,H,W] fp16 -> per-core [BPC,128,SLAB]; slab free = (c, hb, w)
    xs = x16.reshape(NCORES, BPC, C, HB, 128, W).transpose(0, 1, 4, 2, 3, 5)
    return np.ascontiguousarray(xs.reshape(NCORES, BPC, 128, SLAB))


def kernel(pred, target, source, _trace=False):
    from concourse.bass_utils import run_bass_kernel_spmd

    rescale = bool(source.min() < 0)
    nc = _get_nc(rescale)

    p = _host_layout(pred.astype(np.float16))
    t = _host_layout(target.astype(np.float16))
    a = _host_layout(source.astype(np.float16))
    k = _band_matrices()

    in_maps = [
        {"p": p[i], "t": t[i], "a": a[i], "k": k} for i in range(NCORES)
    ]
    res = run_bass_kernel_spmd(
        nc, in_maps, core_ids=list(range(NCORES)), trace=_trace
    )
    parts = np.stack([r["o"] for r in res.results])  # [8,128,NSLOT]
    ps = parts.sum(axis=(0, 1), dtype=np.float64)    # [NSLOT]
    sl = ps[:BPC * HB * 6].reshape(BPC * HB, 6)
    s_m, sd, s_yw, s_ww, s_dw = (sl[:, j].sum() for j in range(5))
    total = (4.0 / (3 * N_TOT)) * sd + (2.0 / (225.0 * N_TOT)) * (
        s_m + 2.0 * s_dw + 0.5 * s_yw - s_ww
    )
    out = np.float32(total)
    if _trace:
        return out, res
    return out
